# revision 5
# baseline (speedup 1.0000x reference)
"""Trainium2 (8 NeuronCores) kernel for batched multi-head causal attention.

Problem: q,k,v [4, 16, 2048, 64] f32, attention_mask [4, 1, 2048] (all ones).
Reference: softmax((q@k^T + causal_mask) * 1/sqrt(64)) @ v.

Sharding: pure data/head parallelism. B*H = 64 heads, 8 heads per core; core c
takes flattened heads [8c, 8c+8).  No cross-core communication.

Design notes (v2 — ScalarE-bound pipeline):
  - The hard floor is the Activation engine: every visible score needs one
    exp (1 elem/cycle/lane @1.2GHz => ~116us/core streaming), so everything
    else is organized to keep ScalarE streaming continuously with as few
    activation instructions as possible (each costs ~185ns of fixed busy
    overhead on top of streaming).
  - Q^T/K^T ([64, 2048] per head, two heads packed on partitions 0:64/64:128)
    are produced by DMA-transpose (XBAR ucode, 16-bit) from bf16 natural
    tiles; the f32->bf16 casts run on DVE.  Neither PE nor ScalarE touches
    the input preprocessing.
  - Scores are computed transposed S^T[s, l] in [128, 1536] f32 PSUM slots
    (3 banks, 2 slots) so one exp instruction covers up to 3 s-tiles.  The
    4 diagonal s-tiles of each l-tile are packed contiguously (widths
    512,384,128,256 -> cols [0,512,896,1024], each within a bank) so the
    whole causal wedge is ONE exp instruction of 1280 cols.
  - Causal masking applied post-exp via gpsimd affine_select (fill 0.0) on
    the diagonal tiles only; QK matmuls for the two heads of a pair sit on
    PE row groups 0:64 / 64:128 (auto tile_position) and are interleaved
    per s-tile so the hardware can run them concurrently.
  - Softmax denominator comes free from an appended ones-column on V
    (PV stationary is [128, 65]); output is computed unnormalized into a
    [65, 512] PSUM accumulator per (head, l-tile), then PE-transposed back
    and scaled by the reciprocal row-sum (DVE).  Output DMA per l-tile.
  - Emission is software-pipelined: QK of stage i+1 is emitted before
    exp/PV of stage i, so ScalarE never waits behind PV in the PE FIFO;
    epilogues are deferred one further stage.
"""

import numpy as np
from contextlib import ExitStack

# problem shape (hardcoded; kernel.py must be self-contained)
B, H, S, D = 4, 16, 2048, 64
NCORES = 8
NH = (B * H) // NCORES   # 8 heads per core
ST = 128                 # s-tile (key) rows per matmul
NST = S // ST            # 16 s-tiles
LT = 512                 # l-tile (query) columns per psum bank
NLT = S // LT            # 4 l-tiles
GRP = 3                  # s-tiles per full exp group (3 psum banks per slot)
SCALE = 1.0 / float(np.sqrt(D))

# Diagonal s-tile packing inside a [128, 1536] score slot: (j, col, width).
# Emission in ascending j; packed order is j = 0,1,3,2 so every matmul
# output [col, col+width) stays inside a single 512-f32 PSUM bank and the
# 1280 used columns are contiguous (one exp instruction).
DIAG = ((0, 0, 512), (1, 512, 384), (2, 1024, 256), (3, 896, 128))

_CACHE = {}


def _stages():
    """Per-pair stage list: ('full', lt, [t...]) / ('diag', lt)."""
    out = []
    for lt in range(NLT):
        n_full = 4 * lt
        if n_full:
            n_groups = -(-n_full // GRP)
            base, rem = divmod(n_full, n_groups)
            start = 0
            for gi in range(n_groups):
                size = base + (1 if gi < rem else 0)
                out.append(("full", lt, list(range(start, start + size))))
                start += size
        out.append(("diag", lt, None))
    return out


def _build_nc(reps=1, bodies=1):
    import concourse.bacc as bacc
    import concourse.mybir as mybir
    import concourse.tile as tile
    from concourse.masks import make_identity

    F32 = mybir.dt.float32
    BF16 = mybir.dt.bfloat16
    EXP = mybir.ActivationFunctionType.Exp

    nc = bacc.Bacc("TRN2", target_bir_lowering=False, debug=False, num_devices=NCORES)

    q_d = nc.dram_tensor("q", [NH, S, D], F32, kind="ExternalInput")
    k_d = nc.dram_tensor("k", [NH, S, D], F32, kind="ExternalInput")
    v_d = nc.dram_tensor("v", [NH, S, D], F32, kind="ExternalInput")
    o_d = nc.dram_tensor("out", [NH, S, D], F32, kind="ExternalOutput")

    with tile.TileContext(nc) as tc, ExitStack() as ctx:
        const = ctx.enter_context(tc.tile_pool(name="const", bufs=1))
        nat = ctx.enter_context(tc.tile_pool(name="nat", bufs=2))
        natv = ctx.enter_context(tc.tile_pool(name="natv", bufs=4))
        qkt = ctx.enter_context(tc.tile_pool(name="qkt", bufs=2))
        pts = ctx.enter_context(tc.tile_pool(name="pts", bufs=4))
        ovs = ctx.enter_context(tc.tile_pool(name="ovs", bufs=2))
        rts = ctx.enter_context(tc.tile_pool(name="rts", bufs=2))
        osb = ctx.enter_context(tc.tile_pool(name="osb", bufs=3))
        psc = ctx.enter_context(tc.tile_pool(name="psc", bufs=2, space="PSUM"))
        ppv = ctx.enter_context(tc.tile_pool(name="ppv", bufs=2, space="PSUM"))

        identb = const.tile([128, 128], mybir.dt.bfloat16, tag="identb")
        make_identity(nc, identb[:])

        import contextlib

        _eng = mybir.EngineType
        loop = (
            tc.For_i(0, reps, 1,
                     hint_engines=(_eng.PE, _eng.DVE, _eng.Activation, _eng.Pool, _eng.SP))
            if reps > 1
            else contextlib.nullcontext()
        )
        with loop:
            for _body_i in range(bodies):
                _emit_body(nc, mybir, F32, BF16, EXP,
                           nat, natv, qkt, pts, ovs, rts, osb,
                           psc, ppv, identb, q_d, k_d, v_d, o_d)

    nc.compile()
    return nc


def _emit_body(nc, mybir, F32, BF16, EXP,
               nat, natv, qkt, pts, ovs, rts, osb,
               psc, ppv, identb, q_d, k_d, v_d, o_d):
    NPAIR = NH // 2
    stages = _stages()
    PROLOG_AT = 4  # stage index of pair p at which pair p+1's inputs start

    def prologue(pair, first=False):
        """DMA loads + DVE casts + DMA-transposes for one head pair.

        For the first pair everything is chunked in units of 4 s-tiles and
        interleaved (q,k,v) so the first QK matmul can start ~6us in instead
        of waiting for whole-tensor loads to clear the DMA queue.
        """
        hA, hB = 2 * pair, 2 * pair + 1
        ctxp = {"heads": (hA, hB), "pvts": {}}
        raws, nbs, vts, rawvs = {}, {}, {}, {}
        for src, tg in ((q_d, "q"), (k_d, "k")):
            raws[tg] = nat.tile([128, NST, 2 * D], F32, tag="raw" + tg,
                                name="raw_" + tg)
            nbs[tg] = nat.tile([128, NST, 2 * D], BF16, tag="nb" + tg,
                               name="nb_" + tg)
            ctxp[tg] = qkt.tile([128, S], BF16, tag="T" + tg, name="T_" + tg)
        for h in (hA, hB):
            rawvs[h] = natv.tile([128, NST, D], F32, tag="rawv", name="raw_v")
            vts[h] = natv.tile([128, NST, D + 1], BF16, tag="vt", name="vt")
        ctxp["v"] = vts

        def load_qk(tg, src, t0, t1):
            raw, nb, tT = raws[tg], nbs[tg], ctxp[tg]
            for i, h in enumerate((hA, hB)):
                nc.sync.dma_start(
                    out=raw[:, t0:t1, i * D: (i + 1) * D],
                    in_=src.ap()[h].rearrange("(t p) d -> p t d", p=128)[:, t0:t1, :],
                )
            nc.vector.tensor_copy(nb[:, t0:t1, :], raw[:, t0:t1, :])
            for t in range(t0, t1):
                nc.sync.dma_start(
                    out=tT[:, ST * t: ST * (t + 1)], in_=nb[:, t, :], transpose=True
                )

        def load_v(t0, t1):
            for h in (hA, hB):
                nc.sync.dma_start(
                    out=rawvs[h][:, t0:t1, :],
                    in_=v_d.ap()[h].rearrange("(t p) d -> p t d", p=128)[:, t0:t1, :],
                )
                nc.vector.tensor_copy(vts[h][:, t0:t1, 0:D], rawvs[h][:, t0:t1, :])
                nc.gpsimd.memset(vts[h][:, t0:t1, D: D + 1], 1.0)

        if first:
            for c in range(4):
                t0, t1 = 4 * c, 4 * c + 4
                load_qk("q", q_d, t0, t1)
                load_qk("k", k_d, t0, t1)
                load_v(t0, t1)
        else:
            load_qk("q", q_d, 0, NST)
            load_qk("k", k_d, 0, NST)
            load_v(0, NST)
        return ctxp

    def emit_qk(st, P):
        kind, lt, arg = st
        l0 = lt * LT
        QT, KT = P["q"], P["k"]
        scs = []
        for _ in range(2):
            sc = psc.tile([128, GRP * LT], F32, tag="sc", name="sc")
            scs.append(sc)
        if kind == "full":
            tiles = [(LT * idx, LT, t) for idx, t in enumerate(arg)]
        else:
            tiles = [(cj, wj, 4 * lt + j) for (j, cj, wj) in DIAG]
        for (col, w, t) in tiles:
            loff = l0 + (LT - w)  # first visible l-column for this s-tile
            for hi, rb in ((0, 0), (1, 64)):
                nc.tensor.matmul(
                    scs[hi][:, col: col + w],
                    lhsT=KT[rb: rb + 64, ST * t: ST * (t + 1)],
                    rhs=QT[rb: rb + 64, loff: l0 + LT],
                    start=True,
                    stop=True,
                )
        return scs

    def finish(st, P, scs):
        kind, lt, arg = st
        hA, hB = P["heads"]
        pvts = P["pvts"]
        if lt not in pvts:
            pvts[lt] = {}
            for h in (hA, hB):
                pvt = ppv.tile([D + 1, LT], F32, tag="pv", name="pvt")
                pvts[lt][h] = pvt
        ncols = LT * len(arg) if kind == "full" else 1280
        ptt = []
        for hi in range(2):
            pt = pts.tile([128, GRP * LT], BF16, tag="pt", name="pt")
            nc.scalar.activation(pt[:, 0:ncols], scs[hi][:, 0:ncols], EXP, scale=SCALE)
            ptt.append(pt)
        if kind == "diag":
            for hi in range(2):
                for (j, cj, wj) in DIAG:
                    nc.gpsimd.affine_select(
                        out=ptt[hi][:, cj: cj + wj],
                        in_=ptt[hi][:, cj: cj + wj],
                        compare_op=mybir.AluOpType.is_ge,
                        fill=0.0,
                        base=0,
                        channel_multiplier=-1,
                        pattern=[[1, wj]],
                    )
        # PV accumulation
        if kind == "full":
            tiles = [(LT * idx, LT, t, 0) for idx, t in enumerate(arg)]
        else:
            tiles = [(cj, wj, 4 * lt + j, ST * j) for (j, cj, wj) in DIAG]
        for (col, w, t, po) in tiles:
            for hi, h in ((0, hA), (1, hB)):
                nc.tensor.matmul(
                    pvts[lt][h][:, po: po + w],
                    lhsT=P["v"][h][:, t, :],
                    rhs=ptt[hi][:, col: col + w],
                    start=(t == 0),
                    stop=(kind == "diag" and t == 4 * lt + 3),
                )
        if kind == "diag":
            return lambda: epilogue(lt, P)
        return None

    def epilogue(lt, P):
        hA, hB = P["heads"]
        for h in (hA, hB):
            pvt = P["pvts"][lt][h]
            ovt = ovs.tile([D + 1, LT], BF16, tag="ovt", name="ovt")
            nc.vector.tensor_copy(ovt[:], pvt[:])
            ost = ppv.tile([128, 4 * (D + 4)], BF16, tag="pv", name="ost")
            for j in range(4):
                nc.tensor.transpose(
                    ost[:, (D + 4) * j: (D + 4) * j + D + 1],
                    ovt[:, 128 * j: 128 * (j + 1)],
                    identb[0: D + 1, 0: D + 1],
                )
            osr = ost[:].rearrange("p (j c) -> p j c", c=D + 4)
            rt = rts.tile([128, 4], F32, tag="rt", name="rt")
            nc.vector.reciprocal(rt[:], osr[:, :, D])
            outsb = osb.tile([128, 4, D], F32, tag="outsb", name="outsb")
            nc.vector.tensor_mul(
                outsb[:], osr[:, :, 0:D],
                rt[:].unsqueeze(2).to_broadcast((128, 4, D)),
            )
            nc.sync.dma_start(
                out=o_d.ap()[h].rearrange("(c p) d -> p c d", p=128)[:, 4 * lt: 4 * lt + 4, :],
                in_=outsb[:],
            )

    P = {0: prologue(0)}
    pend = None       # () -> epilogue-closure | None   (finish of prev stage)
    pend_epi = None   # deferred epilogue
    for pair in range(NPAIR):
        for si, st in enumerate(stages):
            scs = emit_qk(st, P[pair])
            if pend_epi is not None:
                pend_epi()
                pend_epi = None
            if pend is not None:
                pend_epi = pend()
                pend = None
            pend = (lambda st=st, pr=pair, sc=scs: finish(st, P[pr], sc))
            if si == PROLOG_AT and pair + 1 < NPAIR:
                P[pair + 1] = prologue(pair + 1)
    if pend_epi is not None:
        pend_epi()
        pend_epi = None
    if pend is not None:
        pend_epi = pend()
    if pend_epi is not None:
        pend_epi()


def get_nc(reps=1, bodies=1):
    key = (reps, bodies)
    if key not in _CACHE:
        _CACHE[key] = _build_nc(reps, bodies)
    return _CACHE[key]


def make_in_maps(q, k, v):
    q = np.ascontiguousarray(np.asarray(q, dtype=np.float32).reshape(B * H, S, D))
    k = np.ascontiguousarray(np.asarray(k, dtype=np.float32).reshape(B * H, S, D))
    v = np.ascontiguousarray(np.asarray(v, dtype=np.float32).reshape(B * H, S, D))
    maps = []
    for c in range(NCORES):
        sl = slice(c * NH, (c + 1) * NH)
        maps.append(
            {
                "q": np.ascontiguousarray(q[sl]),
                "k": np.ascontiguousarray(k[sl]),
                "v": np.ascontiguousarray(v[sl]),
            }
        )
    return maps


def kernel(q, k, v, attention_mask=None, **_ignored):
    """Full inputs in, full output out. attention_mask is all-ones by
    construction in this problem and drops out of the math."""
    from concourse.bass_utils import run_bass_kernel_spmd

    nc = get_nc()
    res = run_bass_kernel_spmd(nc, make_in_maps(q, k, v), core_ids=list(range(NCORES)))
    out = np.concatenate([res.results[c]["out"] for c in range(NCORES)], axis=0)
    return out.reshape(B, H, S, D).astype(np.float32)


# revision 12
# speedup vs baseline: 1.5755x; 1.5755x over previous
"""Trainium2 (8 NeuronCores) kernel for batched multi-head causal attention.

Problem: q,k,v [4, 16, 2048, 64] f32, attention_mask [4, 1, 2048] (all ones).
Reference: softmax((q@k^T + causal_mask) * 1/sqrt(64)) @ v.

Sharding: pure data/head parallelism. B*H = 64 heads, 8 heads per core; core c
takes flattened heads [8c, 8c+8).  No cross-core communication.

Design notes (v2 — ScalarE-bound pipeline):
  - The hard floor is the Activation engine: every visible score needs one
    exp (1 elem/cycle/lane @1.2GHz => ~116us/core streaming), so everything
    else is organized to keep ScalarE streaming continuously with as few
    activation instructions as possible (each costs ~185ns of fixed busy
    overhead on top of streaming).
  - Q^T/K^T ([64, 2048] per head, two heads packed on partitions 0:64/64:128)
    are produced by DMA-transpose (XBAR ucode, 16-bit) from bf16 natural
    tiles; the f32->bf16 casts run on DVE.  Neither PE nor ScalarE touches
    the input preprocessing.
  - Scores are computed transposed S^T[s, l] in [128, 1536] f32 PSUM slots
    (3 banks, 2 slots) so one exp instruction covers up to 3 s-tiles.  The
    4 diagonal s-tiles of each l-tile are packed contiguously (widths
    512,384,128,256 -> cols [0,512,896,1024], each within a bank) so the
    whole causal wedge is ONE exp instruction of 1280 cols.
  - Causal masking applied post-exp via gpsimd affine_select (fill 0.0) on
    the diagonal tiles only; QK matmuls for the two heads of a pair sit on
    PE row groups 0:64 / 64:128 (auto tile_position) and are interleaved
    per s-tile so the hardware can run them concurrently.
  - Softmax denominator comes free from an appended ones-column on V
    (PV stationary is [128, 65]); output is computed unnormalized into a
    [65, 512] PSUM accumulator per (head, l-tile), then PE-transposed back
    and scaled by the reciprocal row-sum (DVE).  Output DMA per l-tile.
  - Emission is software-pipelined: QK of stage i+1 is emitted before
    exp/PV of stage i, so ScalarE never waits behind PV in the PE FIFO;
    epilogues are deferred one further stage.
"""

import numpy as np
from contextlib import ExitStack

# problem shape (hardcoded; kernel.py must be self-contained)
B, H, S, D = 4, 16, 2048, 64
NCORES = 8
NH = (B * H) // NCORES   # 8 heads per core
ST = 128                 # s-tile (key) rows per matmul
NST = S // ST            # 16 s-tiles
LT = 512                 # l-tile (query) columns per psum bank
NLT = S // LT            # 4 l-tiles
GRP = 3                  # s-tiles per full exp group (3 psum banks per slot)
SCALE = 1.0 / float(np.sqrt(D))

# Diagonal s-tile packing inside a [128, 1536] score slot: (j, col, width).
# Emission in ascending j; packed order is j = 0,1,3,2 so every matmul
# output [col, col+width) stays inside a single 512-f32 PSUM bank and the
# 1280 used columns are contiguous (one exp instruction).
DIAG = ((0, 0, 512), (1, 512, 384), (2, 1024, 256), (3, 896, 128))

_CACHE = {}

# Diagnostic emission mode for compositional HW timing:
#   "full"     - the real kernel
#   "prologue" - only loads + casts + DMA-transposes
#   "noexp"    - prologue + QK matmuls
#   "nopv"     - prologue + QK + exp
#   "noepi"    - everything except epilogue + output DMA
_MODE = "full"

# Q/K transpose mechanism: True = XBAR DMA-transpose, False = PE transpose
# staged through a score-slot PSUM bank + DVE copy (baseline mechanism).
# HW-measured: DmaTransposeAnt costs ~2.3us per [128,128] tile on real
# silicon (vs 112ns in the cost model) - keep False.
_DMA_T = False


def _stages():
    """Per-pair stage list: ('full', lt, [t...]) / ('diag', lt)."""
    out = []
    for lt in range(NLT):
        n_full = 4 * lt
        if n_full:
            n_groups = -(-n_full // GRP)
            base, rem = divmod(n_full, n_groups)
            start = 0
            for gi in range(n_groups):
                size = base + (1 if gi < rem else 0)
                out.append(("full", lt, list(range(start, start + size))))
                start += size
        out.append(("diag", lt, None))
    return out


def _build_nc(reps=1, bodies=1):
    import concourse.bacc as bacc
    import concourse.mybir as mybir
    import concourse.tile as tile
    from concourse.masks import make_identity

    F32 = mybir.dt.float32
    BF16 = mybir.dt.bfloat16
    EXP = mybir.ActivationFunctionType.Exp

    nc = bacc.Bacc("TRN2", target_bir_lowering=False, debug=False, num_devices=NCORES)

    q_d = nc.dram_tensor("q", [NH, S, D], F32, kind="ExternalInput")
    k_d = nc.dram_tensor("k", [NH, S, D], F32, kind="ExternalInput")
    v_d = nc.dram_tensor("v", [NH, S, D], F32, kind="ExternalInput")
    o_d = nc.dram_tensor("out", [NH, S, D], F32, kind="ExternalOutput")

    with tile.TileContext(nc) as tc, ExitStack() as ctx:
        const = ctx.enter_context(tc.tile_pool(name="const", bufs=1))
        nat = ctx.enter_context(tc.tile_pool(name="nat", bufs=2))
        natv = ctx.enter_context(tc.tile_pool(name="natv", bufs=4))
        qkt = ctx.enter_context(tc.tile_pool(name="qkt", bufs=2))
        pts = ctx.enter_context(tc.tile_pool(name="pts", bufs=4))
        ovs = ctx.enter_context(tc.tile_pool(name="ovs", bufs=2))
        rts = ctx.enter_context(tc.tile_pool(name="rts", bufs=2))
        osb = ctx.enter_context(tc.tile_pool(name="osb", bufs=3))
        psc = ctx.enter_context(tc.tile_pool(name="psc", bufs=2, space="PSUM"))
        ppv = ctx.enter_context(tc.tile_pool(name="ppv", bufs=2, space="PSUM"))

        identb = const.tile([128, 128], mybir.dt.bfloat16, tag="identb")
        make_identity(nc, identb[:])

        import contextlib

        _eng = mybir.EngineType
        loop = (
            tc.For_i(0, reps, 1,
                     hint_engines=(_eng.PE, _eng.DVE, _eng.Activation, _eng.Pool, _eng.SP))
            if reps > 1
            else contextlib.nullcontext()
        )
        with loop:
            for _body_i in range(bodies):
                _emit_body(nc, mybir, F32, BF16, EXP,
                           nat, natv, qkt, pts, ovs, rts, osb,
                           psc, ppv, identb, q_d, k_d, v_d, o_d)

    nc.compile()
    return nc


def _emit_body(nc, mybir, F32, BF16, EXP,
               nat, natv, qkt, pts, ovs, rts, osb,
               psc, ppv, identb, q_d, k_d, v_d, o_d):
    NPAIR = NH // 2
    stages = _stages()
    PROLOG_AT = 4  # stage index of pair p at which pair p+1's inputs start

    def prologue(pair, first=False):
        """DMA loads + DVE casts + DMA-transposes for one head pair.

        For the first pair everything is chunked in units of 4 s-tiles and
        interleaved (q,k,v) so the first QK matmul can start ~6us in instead
        of waiting for whole-tensor loads to clear the DMA queue.
        """
        hA, hB = 2 * pair, 2 * pair + 1
        ctxp = {"heads": (hA, hB), "pvts": {}}
        raws, nbs, vts, rawvs = {}, {}, {}, {}
        for src, tg in ((q_d, "q"), (k_d, "k")):
            raws[tg] = nat.tile([128, NST, 2 * D], F32, tag="raw" + tg,
                                name="raw_" + tg)
            nbs[tg] = nat.tile([128, NST, 2 * D], BF16, tag="nb" + tg,
                               name="nb_" + tg)
            ctxp[tg] = qkt.tile([128, S], BF16, tag="T" + tg, name="T_" + tg)
        for h in (hA, hB):
            rawvs[h] = natv.tile([128, NST, D], F32, tag="rawv", name="raw_v")
            vts[h] = natv.tile([128, NST, D + 1], BF16, tag="vt", name="vt")
        ctxp["v"] = vts

        def load_qk(tg, src, t0, t1):
            raw, nb, tT = raws[tg], nbs[tg], ctxp[tg]
            for i, h in enumerate((hA, hB)):
                nc.sync.dma_start(
                    out=raw[:, t0:t1, i * D: (i + 1) * D],
                    in_=src.ap()[h].rearrange("(t p) d -> p t d", p=128)[:, t0:t1, :],
                )
            nc.vector.tensor_copy(nb[:, t0:t1, :], raw[:, t0:t1, :])
            if _DMA_T:
                for t in range(t0, t1):
                    nc.sync.dma_start(
                        out=tT[:, ST * t: ST * (t + 1)], in_=nb[:, t, :],
                        transpose=True,
                    )
            else:
                # PE transpose staged through a score-slot bank; chunks are
                # emitted in even counts so the sc ring parity is preserved.
                for c0 in range(t0, t1, 8):
                    n = min(8, t1 - c0)
                    stg = psc.tile([128, GRP * LT], BF16, tag="sc", name="stg")
                    for j in range(n):
                        nc.tensor.transpose(
                            stg[:, 128 * j: 128 * (j + 1)], nb[:, c0 + j, :],
                            identb[:],
                        )
                    nc.vector.tensor_copy(
                        tT[:, ST * c0: ST * (c0 + n)], stg[:, 0: 128 * n]
                    )

        def load_v(t0, t1):
            for h in (hA, hB):
                nc.sync.dma_start(
                    out=rawvs[h][:, t0:t1, :],
                    in_=v_d.ap()[h].rearrange("(t p) d -> p t d", p=128)[:, t0:t1, :],
                )
                nc.vector.tensor_copy(vts[h][:, t0:t1, 0:D], rawvs[h][:, t0:t1, :])
                nc.gpsimd.memset(vts[h][:, t0:t1, D: D + 1], 1.0)

        if first:
            for c in range(4):
                t0, t1 = 4 * c, 4 * c + 4
                load_qk("q", q_d, t0, t1)
                load_qk("k", k_d, t0, t1)
                load_v(t0, t1)
        else:
            load_qk("q", q_d, 0, NST)
            load_qk("k", k_d, 0, NST)
            load_v(0, NST)
        return ctxp

    def emit_qk(st, P):
        kind, lt, arg = st
        l0 = lt * LT
        QT, KT = P["q"], P["k"]
        scs = []
        for _ in range(2):
            sc = psc.tile([128, GRP * LT], F32, tag="sc", name="sc")
            scs.append(sc)
        if kind == "full":
            tiles = [(LT * idx, LT, t) for idx, t in enumerate(arg)]
        else:
            tiles = [(cj, wj, 4 * lt + j) for (j, cj, wj) in DIAG]
        for (col, w, t) in tiles:
            loff = l0 + (LT - w)  # first visible l-column for this s-tile
            for hi, rb in ((0, 0), (1, 64)):
                nc.tensor.matmul(
                    scs[hi][:, col: col + w],
                    lhsT=KT[rb: rb + 64, ST * t: ST * (t + 1)],
                    rhs=QT[rb: rb + 64, loff: l0 + LT],
                    start=True,
                    stop=True,
                )
        return scs

    def finish(st, P, scs):
        kind, lt, arg = st
        hA, hB = P["heads"]
        pvts = P["pvts"]
        if lt not in pvts:
            pvts[lt] = {}
            for h in (hA, hB):
                pvt = ppv.tile([D + 1, LT], F32, tag="pv", name="pvt")
                pvts[lt][h] = pvt
        ncols = LT * len(arg) if kind == "full" else 1280
        ptt = []
        for hi in range(2):
            pt = pts.tile([128, GRP * LT], BF16, tag="pt", name="pt")
            nc.scalar.activation(pt[:, 0:ncols], scs[hi][:, 0:ncols], EXP, scale=SCALE)
            ptt.append(pt)
        if _MODE == "nopv":
            return None
        if kind == "diag":
            for hi in range(2):
                for (j, cj, wj) in DIAG:
                    nc.gpsimd.affine_select(
                        out=ptt[hi][:, cj: cj + wj],
                        in_=ptt[hi][:, cj: cj + wj],
                        compare_op=mybir.AluOpType.is_ge,
                        fill=0.0,
                        base=0,
                        channel_multiplier=-1,
                        pattern=[[1, wj]],
                    )
        # PV accumulation
        if kind == "full":
            tiles = [(LT * idx, LT, t, 0) for idx, t in enumerate(arg)]
        else:
            tiles = [(cj, wj, 4 * lt + j, ST * j) for (j, cj, wj) in DIAG]
        for (col, w, t, po) in tiles:
            for hi, h in ((0, hA), (1, hB)):
                nc.tensor.matmul(
                    pvts[lt][h][:, po: po + w],
                    lhsT=P["v"][h][:, t, :],
                    rhs=ptt[hi][:, col: col + w],
                    start=(t == 0),
                    stop=(kind == "diag" and t == 4 * lt + 3),
                )
        if kind == "diag" and _MODE != "noepi":
            return lambda: epilogue(lt, P)
        return None

    def epilogue(lt, P):
        hA, hB = P["heads"]
        for h in (hA, hB):
            pvt = P["pvts"][lt][h]
            ovt = ovs.tile([D + 1, LT], BF16, tag="ovt", name="ovt")
            nc.vector.tensor_copy(ovt[:], pvt[:])
            ost = ppv.tile([128, 4 * (D + 4)], BF16, tag="pv", name="ost")
            for j in range(4):
                nc.tensor.transpose(
                    ost[:, (D + 4) * j: (D + 4) * j + D + 1],
                    ovt[:, 128 * j: 128 * (j + 1)],
                    identb[0: D + 1, 0: D + 1],
                )
            osr = ost[:].rearrange("p (j c) -> p j c", c=D + 4)
            rt = rts.tile([128, 4], F32, tag="rt", name="rt")
            nc.vector.reciprocal(rt[:], osr[:, :, D])
            outsb = osb.tile([128, 4, D], F32, tag="outsb", name="outsb")
            nc.vector.tensor_mul(
                outsb[:], osr[:, :, 0:D],
                rt[:].unsqueeze(2).to_broadcast((128, 4, D)),
            )
            nc.sync.dma_start(
                out=o_d.ap()[h].rearrange("(c p) d -> p c d", p=128)[:, 4 * lt: 4 * lt + 4, :],
                in_=outsb[:],
            )

    P = {0: prologue(0, first=True)}
    if _MODE == "prologue":
        for pair in range(1, NPAIR):
            P[pair] = prologue(pair)
        return
    pend = None       # () -> epilogue-closure | None   (finish of prev stage)
    pend_epi = None   # deferred epilogue
    for pair in range(NPAIR):
        for si, st in enumerate(stages):
            scs = emit_qk(st, P[pair])
            if pend_epi is not None:
                pend_epi()
                pend_epi = None
            if pend is not None and _MODE != "noexp":
                pend_epi = pend()
                pend = None
            pend = (lambda st=st, pr=pair, sc=scs: finish(st, P[pr], sc))
            if si == PROLOG_AT and pair + 1 < NPAIR:
                P[pair + 1] = prologue(pair + 1)
    if _MODE == "noexp":
        return
    if pend_epi is not None:
        pend_epi()
        pend_epi = None
    if pend is not None:
        pend_epi = pend()
    if pend_epi is not None:
        pend_epi()


def get_nc(reps=1, bodies=1):
    key = (reps, bodies)
    if key not in _CACHE:
        _CACHE[key] = _build_nc(reps, bodies)
    return _CACHE[key]


def make_in_maps(q, k, v):
    q = np.ascontiguousarray(np.asarray(q, dtype=np.float32).reshape(B * H, S, D))
    k = np.ascontiguousarray(np.asarray(k, dtype=np.float32).reshape(B * H, S, D))
    v = np.ascontiguousarray(np.asarray(v, dtype=np.float32).reshape(B * H, S, D))
    maps = []
    for c in range(NCORES):
        sl = slice(c * NH, (c + 1) * NH)
        maps.append(
            {
                "q": np.ascontiguousarray(q[sl]),
                "k": np.ascontiguousarray(k[sl]),
                "v": np.ascontiguousarray(v[sl]),
            }
        )
    return maps


def kernel(q, k, v, attention_mask=None, **_ignored):
    """Full inputs in, full output out. attention_mask is all-ones by
    construction in this problem and drops out of the math."""
    from concourse.bass_utils import run_bass_kernel_spmd

    nc = get_nc()
    res = run_bass_kernel_spmd(nc, make_in_maps(q, k, v), core_ids=list(range(NCORES)))
    out = np.concatenate([res.results[c]["out"] for c in range(NCORES)], axis=0)
    return out.reshape(B, H, S, D).astype(np.float32)
